# revision 6
# baseline (speedup 1.0000x reference)
"""MaxPool3d (kernel=3, stride=2, padding=1) on Trainium2, 8 NeuronCores.

Input  x: (2, 32, 128, 128, 128) f32  ->  Output: (2, 32, 64, 64, 64) f32.

Compute runs in bf16 (max-pooling is exact in bf16 once inputs are rounded;
worst-case rel err ~2^-9, far inside the 2e-2 gate). The host pre-casts to
bf16 and de-interleaves even/odd w columns so every DVE operand is packed
(innermost stride 1), unlocking the 2x 16-bit DVE mode. This also halves all
HBM traffic vs f32.

Sharding: the 64 (b, c) slices are data-parallel; each of the 8 cores gets 8
slices, processed as 4 slice-pairs (a pair packs 2 slices into the 128 SBUF
partitions).

Per-core algorithm (separable max pooling W -> H -> D):
  - HBM layout per slice (host-prepped): [d, pw, h, w2] bf16 where
    w = 2*w2 + pw. Even-d rows load into slab E (partition 64*s + d/2),
    odd-d into slab O, so the final D-axis pooling is partition-aligned.
  - W pool: T = max(x_even_w, x_odd_w); T[..., 1:] |= x_odd_w[..., :-1]
    (both DVE, 2x mode: all operands packed bf16).
  - H pool: G = max(T[0::2], T[1::2]) on DVE (2x); the odd-row term
    G[1:] |= T[1:-2:2] runs on GpSimd to offload DVE.
  - D pool (partition axis): Et |= G_O on DVE; the 2*od-1 term comes from
    a partition-shifted SBUF->SBUF DMA copy of G_O plus a GpSimd max.

Engine balance per steady chunk: DVE ~6 us, GpSimd ~5 us, DMA ~8 us
(2 MiB loads + 0.25 store + 0.25 shift across both HWDGE rings).
"""

import sys

sys.path.insert(0, "/opt/trn_rl_repo")

import numpy as np
import ml_dtypes

# Shapes (hardcoded per problem spec)
B, C, D, H, W = 2, 32, 128, 128, 128
OD, OH, OW = 64, 64, 64
W2 = W // 2
N_CORES = 8
SLICES_PER_CORE = (B * C) // N_CORES  # 8
PAIRS = SLICES_PER_CORE // 2  # 4
HC = 32  # max h rows per load chunk (tile size)
# ramp-friendly schedule: small first chunks (pair 0 only) so DVE starts early
CHUNK_SIZES_RAMP = [8, 24, 32, 32, 32]
CHUNK_SIZES_STEADY = [32, 32, 32, 32]
assert sum(CHUNK_SIZES_RAMP) == H and max(CHUNK_SIZES_RAMP) == HC
assert sum(CHUNK_SIZES_STEADY) == H

_cache = {}


def _build():
    import concourse.mybir as mybir
    from concourse import bacc
    from concourse.tile import TileContext

    bf16 = mybir.dt.bfloat16
    nc = bacc.Bacc()
    x_ext = nc.declare_dram_parameter(
        "x_shard", [SLICES_PER_CORE, D, 2, H, W2], bf16, isOutput=False
    )
    y_ext = nc.declare_dram_parameter(
        "y_shard", [SLICES_PER_CORE, OD, OH, OW], bf16, isOutput=True
    )

    with TileContext(nc) as tc:
        with (
            tc.tile_pool(name="xpool", bufs=4) as xpool,
            tc.tile_pool(name="fpool", bufs=3) as fpool,
            tc.tile_pool(name="gpool", bufs=3) as gpool,
            tc.tile_pool(name="gspool", bufs=3) as gspool,
            tc.tile_pool(name="opool", bufs=2) as opool,
        ):
            for p in range(PAIRS):
                s0 = 2 * p
                # H pool: slab E accumulates into Et (global rows); slab O
                # goes to a per-chunk Go tile (local rows)
                Et = opool.tile([128, OH, OW], bf16, name="Et", tag="Et")
                pending = None
                Tprev = {0: None, 1: None}
                h0 = 0
                sizes = CHUNK_SIZES_RAMP if p == 0 else CHUNK_SIZES_STEADY
                for c, hc in enumerate(sizes):
                    oh0 = h0 // 2
                    ohc = hc // 2
                    ohr = slice(oh0, oh0 + ohc)
                    # ---- both slab loads first; each HWDGE ring carries only
                    # loads so it streams ahead, never compute-gated ----
                    xts = {}
                    for par, name, eng in ((0, "E", nc.sync), (1, "O", nc.sync)):
                        xt = xpool.tile(
                            [128, 2, HC, W2], bf16, name=f"x{name}", tag=f"x{name}"
                        )
                        eng.dma_start(
                            out=xt[:, :, 0:hc, :],
                            in_=x_ext[s0 : s0 + 2, par : D : 2, :, h0 : h0 + hc, :],
                        )
                        xts[par] = xt
                    Go = None
                    for par, name in ((0, "E"), (1, "O")):
                        if par == 1 and pending is not None:
                            # ---- software-pipelined D pool of the PREVIOUS
                            # chunk: runs while this chunk's shift DMA is in
                            # flight, hiding SWDGE latency off DVE's path ----
                            pGo, pGs, pohr = pending
                            nc.vector.tensor_max(
                                out=Et[:, pohr, :], in0=Et[:, pohr, :],
                                in1=pGo,
                            )
                            nc.vector.tensor_max(
                                out=Et[:, pohr, :], in0=Et[:, pohr, :],
                                in1=pGs,
                            )
                            nc.scalar.dma_start(
                                out=y_ext[s0 : s0 + 2, :, pohr, :],
                                in_=Et[:, pohr, :],
                            )
                            pending = None
                        xt = xts[par]
                        # ---- W pool into per-chunk T tile (all packed: 2x) ----
                        Tt = fpool.tile(
                            [128, HC, OW], bf16, name=f"T{name}", tag=f"T{name}"
                        )
                        nc.vector.tensor_max(
                            out=Tt[:, 0:hc, :],
                            in0=xt[:, 0, 0:hc, :],
                            in1=xt[:, 1, 0:hc, :],
                        )
                        nc.vector.tensor_max(
                            out=Tt[:, 0:hc, 1:OW],
                            in0=Tt[:, 0:hc, 1:OW],
                            in1=xt[:, 1, 0:hc, 0 : W2 - 1],
                        )
                        # ---- H pool rows of this chunk ----
                        if par == 0:
                            Gt, g0 = Et, oh0
                        else:
                            Go = gpool.tile(
                                [128, HC // 2, OW], bf16, name="Go", tag="Go"
                            )
                            Gt, g0 = Go, 0
                        nc.vector.tensor_max(
                            out=Gt[:, g0 : g0 + ohc, :],
                            in0=Tt[:, 0:hc:2, :],
                            in1=Tt[:, 1:hc:2, :],
                        )
                        nc.vector.tensor_max(
                            out=Gt[:, g0 + 1 : g0 + ohc, :],
                            in0=Gt[:, g0 + 1 : g0 + ohc, :],
                            in1=Tt[:, 1 : hc - 2 : 2, :],
                        )
                        if c > 0:
                            # boundary row: h = 2*oh0 - 1 = prev chunk's last row
                            nc.vector.tensor_max(
                                out=Gt[:, g0 : g0 + 1, :],
                                in0=Gt[:, g0 : g0 + 1, :],
                                in1=Tprev[par],
                            )
                        Tprev[par] = Tt[:, hc - 1 : hc, :]

                    # ---- launch this chunk's partition shift (d axis) via
                    # the scalar HWDGE ring; the D pool itself is deferred into the next
                    # chunk's compute. Second call overwrites rows 0/64 with
                    # idempotent unshifted values (row 64 must not leak across
                    # slices).
                    Gs = gspool.tile([128, HC // 2, OW], bf16, name="Gs", tag="Gs")
                    nc.scalar.dma_start(
                        out=Gs[1:128, 0:ohc, :], in_=Go[0:127, 0:ohc, :]
                    )
                    nc.scalar.dma_start(
                        out=Gs[0:65:64, 0:ohc, :], in_=Go[0:65:64, 0:ohc, :]
                    )
                    pending = (Go[:, 0:ohc, :], Gs[:, 0:ohc, :], ohr)
                    h0 += hc
                # ---- pair tail: flush the last chunk's D pool + store ----
                pGo, pGs, pohr = pending
                nc.vector.tensor_max(
                    out=Et[:, pohr, :], in0=Et[:, pohr, :], in1=pGo
                )
                nc.vector.tensor_max(
                    out=Et[:, pohr, :], in0=Et[:, pohr, :], in1=pGs
                )
                nc.scalar.dma_start(
                    out=y_ext[s0 : s0 + 2, :, pohr, :], in_=Et[:, pohr, :]
                )
                pending = None
    nc.compile()
    return nc


def _get_nc():
    if "nc" not in _cache:
        _cache["nc"] = _build()
    return _cache["nc"]


def _prep(x: np.ndarray) -> np.ndarray:
    """f32 (B,C,D,H,W) -> bf16 (B*C, D, 2, H, W2): even/odd w de-interleave."""
    xr = np.asarray(x, dtype=np.float32).reshape(B * C, D, H, W2, 2)
    return xr.transpose(0, 1, 4, 2, 3).astype(ml_dtypes.bfloat16)


def run(x: np.ndarray, **spmd_kwargs):
    """Run the SPMD kernel; returns the BassKernelResults (for tracing)."""
    from concourse.bass_utils import run_bass_kernel_spmd

    nc = _get_nc()
    xs = _prep(x)
    in_maps = [
        {"x_shard": np.ascontiguousarray(xs[SLICES_PER_CORE * i : SLICES_PER_CORE * (i + 1)])}
        for i in range(N_CORES)
    ]
    return run_bass_kernel_spmd(nc, in_maps, list(range(N_CORES)), **spmd_kwargs)


def kernel(x: np.ndarray) -> np.ndarray:
    res = run(x)
    out = np.stack([res.results[i]["y_shard"] for i in range(N_CORES)])
    return out.reshape(B, C, OD, OH, OW).astype(np.float32)
